# revision 17
# baseline (speedup 1.0000x reference)
"""GNN message-passing layer (GAT-isotropic) on 8 Trainium2 NeuronCores.

Distribution: dst-nodes sharded across the 8 cores (graph partitioning by
destination); each core gathers remote src features directly from its full
DRAM copy of h (the halo exchange is subsumed by replicating h, which is
small), aggregates per 128-node dst tile with one-hot matmuls into PSUM,
all-reduces the batchnorm statistics across cores, and writes its output
shard. BN scale/shift are folded into per-head affine coefficients.
"""

import contextlib
import ctypes
import sys
import types

import numpy as np

# ---------------------------------------------------------------- axon shim
_SO_PATH = "/opt/axon/libaxon_pjrt.so"


def _make_ntff_hook():
    try:
        lib = ctypes.CDLL(_SO_PATH)
    except OSError:
        return None
    if not hasattr(lib, "axon_start_nrt_profile"):
        return None
    lib.axon_start_nrt_profile.argtypes = [
        ctypes.POINTER(ctypes.c_int64),
        ctypes.c_size_t,
    ]
    lib.axon_start_nrt_profile.restype = ctypes.c_int64
    lib.axon_stop_nrt_profile.argtypes = [ctypes.c_char_p]
    lib.axon_stop_nrt_profile.restype = ctypes.c_int64

    @contextlib.contextmanager
    def _hook(output_dir, device_ids):
        import jax

        jax.devices()
        if device_ids:
            ids = (ctypes.c_int64 * len(device_ids))(*device_ids)
            rc = lib.axon_start_nrt_profile(ids, len(device_ids))
        else:
            rc = lib.axon_start_nrt_profile(None, 0)
        if rc != 0:
            raise RuntimeError(f"axon_start_nrt_profile rc={rc}")
        try:
            yield
        finally:
            n = lib.axon_stop_nrt_profile(str(output_dir).encode())
            print(f"ntff profile: {n} file(s) -> {output_dir}", file=sys.stderr)

    return _hook


def _install_hook_shim():
    if "antenv.axon_hooks" in sys.modules:
        return
    try:
        import antenv
    except ImportError:
        return
    mod = types.ModuleType("antenv.axon_hooks")
    hook = _make_ntff_hook()
    mod.get_axon_ntff_profile_hook = lambda: hook
    mod.set_axon_ntff_profile_hook = lambda h: None
    sys.modules["antenv.axon_hooks"] = mod
    antenv.axon_hooks = mod


_install_hook_shim()

# ---------------------------------------------------------------- constants
P = 128
D = 64
H = 4
N_CORES = 8
EPS = 1e-5
GCH = 8  # chunks per dma_gather group (1024 idxs = SWDGE ring limit)

_BUILD_CACHE = {}


def _host_prep(h, norm, gamma, beta, src, dst, n_tiles, trim=False):
    """Shard + pad edges, build all per-core device input arrays."""
    n = h.shape[0]
    nc_nodes = n_tiles * P
    nodes_pad = N_CORES * nc_nodes
    npairs = nodes_pad // 2
    assert npairs - 1 < 2**15

    src = np.asarray(src).astype(np.int64)
    dst = np.asarray(dst).astype(np.int64)
    e = src.shape[0]
    normf = np.asarray(norm, np.float32).reshape(-1)

    g_tile = dst // P  # global dst tile
    n_gtiles = N_CORES * n_tiles
    gt_cnt = np.bincount(g_tile, minlength=n_gtiles)
    order_t = np.argsort(-gt_cnt, kind="stable")
    assign_core = np.empty(n_gtiles, np.int64)
    assign_slot = np.empty(n_gtiles, np.int64)
    assign_core[order_t] = np.arange(n_gtiles) % N_CORES
    assign_slot[order_t] = np.arange(n_gtiles) // N_CORES
    core = assign_core[g_tile]
    tile_ic = assign_slot[g_tile]
    hlf = (src & 1).astype(np.int64)  # parity of src selects pair half
    dloc = (dst % P).astype(np.float32)

    key = (core * n_tiles + tile_ic) * 2 + hlf
    order = np.argsort(key, kind="stable")
    key_s = key[order]
    counts = np.bincount(key, minlength=N_CORES * n_tiles * 2)
    n_chunks = np.ceil(
        counts.reshape(N_CORES, n_tiles, 2).max(axis=0) / P
    ).astype(np.int64)
    n_chunks = np.maximum(n_chunks, 1)  # [n_tiles, 2]

    # chunk stream order: (tile, parity) lexicographic
    flat_counts = n_chunks.reshape(-1)  # [t0p0, t0p1, t1p0, ...]
    ct = int(flat_counts.sum())
    c0 = ct  # unused (kept for meta compat)
    cum01 = np.concatenate([[0], np.cumsum(flat_counts)])[:-1]  # start chunk of (t,p)

    totslot = ct * P
    pad_idx = -1 if trim else 0
    idx_all = np.full((N_CORES, totslot), pad_idx, np.int16)
    dloc_all = np.full((N_CORES, totslot), -1.0, np.float32)
    w_all = np.zeros((N_CORES, totslot), np.float32)

    grp_start = np.concatenate([[0], np.cumsum(counts)])
    pos_in_grp = np.arange(e) - grp_start[key_s]
    c_s = core[order]
    t_s = tile_ic[order]
    h_s = hlf[order]
    src_s = src[order]
    slot_base_chunks = np.where(h_s == 0, cum01[2 * t_s], cum01[2 * t_s + 1])
    slot = slot_base_chunks * P + pos_in_grp
    idx_all[c_s, slot] = (src_s >> 1).astype(np.int16)
    dloc_all[c_s, slot] = dloc[order]
    w_all[c_s, slot] = normf[src_s]

    import ml_dtypes

    # host-built one-hot: oh[slot, j] = w[slot] * (j == dloc[slot]);
    # DRAM layout [P, ct*P] with [p, ch*P + j] for edge slot ch*P+p
    oh_cols = []
    for c in range(N_CORES):
        oh = np.zeros((totslot, P), np.float32)
        valid = dloc_all[c] >= 0
        rows = np.nonzero(valid)[0]
        oh[rows, dloc_all[c][rows].astype(np.int64)] = w_all[c][rows]
        oh_cols.append(np.ascontiguousarray(
            oh.reshape(ct, P, P).transpose(1, 0, 2).reshape(P, ct * P)
            .astype(ml_dtypes.bfloat16)))

    # gather groups: plain GCH-chunk stripes over the chunk stream
    groups = []  # (has_pads, start_chunk, n_chunks, idx_col_off)
    off = 0
    gs = 0
    while gs < ct:
        gn = min(GCH, ct - gs)
        groups.append((0, gs, gn, off))
        off += gn * P // 16
        gs += gn
    idx_cols = off

    idx_sb_np = []
    for c in range(N_CORES):
        blocks = []
        for s, gs, gn, _ in groups:
            flat = idx_all[c, gs * P : (gs + gn) * P]
            wr = flat.reshape(gn * P // 16, 16).T  # [16, gn*8]
            blocks.append(np.tile(wr, (8, 1)))  # [128, gn*8]
        idx_sb_np.append(np.concatenate(blocks, axis=1))

    # norm of dst nodes, [P, n_tiles] per core (tile slots per assignment)
    norm_pad = np.zeros(nodes_pad, np.float32)
    norm_pad[:n] = normf
    ndst = []
    for c in range(N_CORES):
        nd = np.zeros((P, n_tiles), np.float32)
        for g in range(n_gtiles):
            if assign_core[g] == c:
                nd[:, assign_slot[g]] = norm_pad[g * P : (g + 1) * P]
        ndst.append(nd)

    h_pad = np.zeros((nodes_pad, D), np.float32)
    h_pad[:n] = np.asarray(h, np.float32)
    h_pair = np.ascontiguousarray(
        h_pad.reshape(npairs, 2 * D).astype(ml_dtypes.bfloat16))

    onesc = np.ones((P, 1), np.float32)
    onesr = np.ones((1, P), np.float32)
    gbcat = np.concatenate(
        [np.asarray(gamma, np.float32).reshape(1, H * D),
         np.asarray(beta, np.float32).reshape(1, H * D)],
        axis=1,
    )

    in_maps = []
    for c in range(N_CORES):
        in_maps.append(
            {
                "hp": h_pair,
                "idx": idx_sb_np[c],
                "oh": oh_cols[c],
                "ndst": ndst[c],
                "onesc": onesc,
                "onesr": onesr,
                "gbcat": gbcat,
            }
        )

    meta = {
        "n_tiles": n_tiles,
        "n": n,
        "n_chunks": tuple(map(tuple, n_chunks.tolist())),
        "groups": tuple(groups),
        "ct": ct,
        "idx_cols": idx_cols,
        "cum01": tuple(cum01.tolist()),
        "nodes_pad": nodes_pad,
        "npairs": npairs,
        "trim": bool(trim),
        "assign_core": tuple(assign_core.tolist()),
        "assign_slot": tuple(assign_slot.tolist()),
    }
    return in_maps, meta


def _build_nc(meta):
    from contextlib import ExitStack

    import concourse.tile as tile
    from concourse import bacc, library_config, mybir

    f32 = mybir.dt.float32
    i16 = mybir.dt.int16
    AF = mybir.ActivationFunctionType
    OP = mybir.AluOpType

    bf16 = mybir.dt.bfloat16

    n_tiles = meta["n_tiles"]
    n = meta["n"]
    n_chunks = meta["n_chunks"]
    groups = meta["groups"]
    ct = meta["ct"]
    idx_cols = meta["idx_cols"]
    cum01 = meta["cum01"]
    npairs = meta["npairs"]
    nc_nodes = n_tiles * P
    NQ = 4

    nc = bacc.Bacc("TRN2", target_bir_lowering=False, debug=False,
                   num_devices=N_CORES, num_swdge_queues=NQ)

    h_d = nc.dram_tensor("hp", [npairs, 2 * D], bf16, kind="ExternalInput").ap()
    idx_d = nc.dram_tensor("idx", [P, idx_cols], i16, kind="ExternalInput").ap()
    oh_d = nc.dram_tensor("oh", [P, ct * P], bf16, kind="ExternalInput").ap()
    nd_d = nc.dram_tensor("ndst", [P, n_tiles], f32, kind="ExternalInput").ap()
    onesc_d = nc.dram_tensor("onesc", [P, 1], f32, kind="ExternalInput").ap()
    onesr_d = nc.dram_tensor("onesr", [1, P], f32, kind="ExternalInput").ap()
    gb_d = nc.dram_tensor("gbcat", [1, 2 * H * D], f32, kind="ExternalInput").ap()
    out_d = nc.dram_tensor("out", [nc_nodes, H * D], f32, kind="ExternalOutput").ap()
    cc_in = nc.dram_tensor("cc_in", [1, 2 * D], f32, kind="Internal").ap()
    cc_out = nc.dram_tensor("cc_out", [1, 2 * D], f32, kind="Internal",
                            addr_space="Shared").ap()

    # group lookup: for a global chunk id, which (group index) covers it
    grp_of_chunk = {}
    for gi, (s, gs, gn, off) in enumerate(groups):
        for j in range(gn):
            grp_of_chunk[gs + j] = gi

    with tile.TileContext(nc) as tc, ExitStack() as ctx:
        const = ctx.enter_context(tc.tile_pool(name="const", bufs=1))
        msgp = ctx.enter_context(tc.tile_pool(name="msg", bufs=32))
        ohp = ctx.enter_context(tc.tile_pool(name="oh", bufs=20))
        psp = ctx.enter_context(tc.tile_pool(name="psum", bufs=4, space="PSUM"))
        stp = ctx.enter_context(tc.tile_pool(name="stps", bufs=1, space="PSUM"))
        bcp = ctx.enter_context(tc.tile_pool(name="bc", bufs=1, space="PSUM"))
        aggp = ctx.enter_context(tc.tile_pool(name="agg", bufs=1))
        sqp = ctx.enter_context(tc.tile_pool(name="sq", bufs=2))
        outp = ctx.enter_context(tc.tile_pool(name="outp", bufs=3))
        smal = ctx.enter_context(tc.tile_pool(name="small", bufs=1))

        nc.gpsimd.load_library(library_config.mlp)

        idx_sb = const.tile([P, idx_cols], i16)
        qcol = (idx_cols // 4 + 15) & ~15
        for qi in range(4):
            lo = qi * qcol
            hi = min(idx_cols, lo + qcol)
            if lo < hi:
                nc.sync.dma_start(idx_sb[:, lo:hi], idx_d[:, lo:hi])
        nd_sb = const.tile([P, n_tiles], f32)
        nc.sync.dma_start(nd_sb[:], nd_d[:])
        onesc_sb = const.tile([P, 1], f32)
        nc.sync.dma_start(onesc_sb[:], onesc_d[:])
        onesr_sb = const.tile([1, P], f32)
        nc.sync.dma_start(onesr_sb[:], onesr_d[:])
        gb_sb = const.tile([1, 2 * H * D], f32)
        nc.sync.dma_start(gb_sb[:], gb_d[:])

        agg_all = aggp.tile([P, n_tiles * D], f32)
        sum_ps = stp.tile([1, D], f32)
        sq_ps = stp.tile([1, D], f32)

        emitted = {}

        def get_msg(chg):
            gi = grp_of_chunk[chg]
            if gi not in emitted:
                has_pads, gs, gn, off = groups[gi]
                m = msgp.tile([P, GCH, 2 * D], bf16, tag="msg")
                if has_pads and meta["trim"]:
                    # trailing idxs may be trimmed (-1): zero the tile so
                    # untouched slots stay finite (one-hot zeroes them out)
                    nc.vector.memset(m[:, :gn, :], 0.0)
                nc.gpsimd.dma_gather(
                    out_ap=m[:, :gn, :],
                    in_ap=h_d[:],
                    idxs_ap=idx_sb[:, off : off + gn * P // 16],
                    num_idxs=gn * P,
                    num_idxs_reg=gn * P,
                    elem_size=2 * D,
                    queue_num=gi % NQ,
                )
                o = ohp.tile([P, GCH * P], bf16, tag="oht")
                nc.scalar.dma_start(o[:, : gn * P], oh_d[:, gs * P : (gs + gn) * P])
                emitted[gi] = (m, o)
            has_pads, gs, gn, off = groups[gi]
            return emitted[gi], chg - gs

        for t in range(n_tiles):
            agg_ps = psp.tile([P, D], f32)
            nmm = n_chunks[t][0] + n_chunks[t][1]
            k = 0
            for hh in (0, 1):
                base = cum01[2 * t + hh]
                for j in range(n_chunks[t][hh]):
                    chg = base + j
                    (m, o), slot = get_msg(chg)
                    nc.tensor.matmul(
                        agg_ps[:], lhsT=o[:, slot * P : (slot + 1) * P],
                        rhs=m[:, slot, hh * D : (hh + 1) * D],
                        start=(k == 0), stop=(k == nmm - 1),
                    )
                    k += 1
            a_sl = agg_all[:, t * D : (t + 1) * D]
            nc.vector.tensor_copy(a_sl, agg_ps[:])
            sq = sqp.tile([P, D], f32)
            nc.vector.tensor_tensor(sq[:], a_sl, a_sl, op=OP.mult)
            nc.tensor.matmul(sum_ps[:], lhsT=onesc_sb[:], rhs=a_sl,
                             start=(t == 0), stop=(t == n_tiles - 1),
                             skip_group_check=True)
            nc.tensor.matmul(sq_ps[:], lhsT=onesc_sb[:], rhs=sq[:],
                             start=(t == 0), stop=(t == n_tiles - 1),
                             skip_group_check=True)

        # ---- batchnorm statistics all-reduce ----
        st_sb = smal.tile([1, 2 * D], f32)
        nc.scalar.activation(st_sb[:, 0:D], sum_ps[:], AF.Copy)
        nc.scalar.activation(st_sb[:, D : 2 * D], sq_ps[:], AF.Copy)
        nc.sync.dma_start(cc_in[:], st_sb[:])
        nc.gpsimd.collective_compute(
            "AllReduce", OP.add,
            replica_groups=[list(range(N_CORES))],
            ins=[cc_in[:]], outs=[cc_out[:]],
        )
        gs_sb = smal.tile([1, 2 * D], f32)
        nc.sync.dma_start(gs_sb[:], cc_out[:])

        mean = smal.tile([1, D], f32)
        nc.vector.tensor_scalar_mul(mean[:], gs_sb[:, 0:D], 1.0 / n)
        ex2 = smal.tile([1, D], f32)
        nc.vector.tensor_scalar_mul(ex2[:], gs_sb[:, D : 2 * D], 1.0 / n)
        var = smal.tile([1, D], f32)
        nc.vector.tensor_tensor(var[:], mean[:], mean[:], op=OP.mult)
        nc.vector.tensor_tensor(var[:], ex2[:], var[:], op=OP.subtract)
        nc.vector.tensor_scalar_add(var[:], var[:], EPS)
        sd = smal.tile([1, D], f32)
        nc.scalar.activation(sd[:], var[:], AF.Sqrt)
        rstd = smal.tile([1, D], f32)
        nc.vector.reciprocal(rstd[:], sd[:])

        ab = smal.tile([1, 2 * H * D], f32)
        for hh in range(H):
            asl = ab[:, hh * D : (hh + 1) * D]
            nc.vector.tensor_tensor(asl, gb_sb[:, hh * D : (hh + 1) * D],
                                    rstd[:], op=OP.mult)
        hd = H * D
        for hh in range(H):
            bsl = ab[:, hd + hh * D : hd + (hh + 1) * D]
            nc.vector.tensor_tensor(bsl, ab[:, hh * D : (hh + 1) * D],
                                    mean[:], op=OP.mult)
            nc.vector.tensor_tensor(
                bsl, gb_sb[:, hd + hh * D : hd + (hh + 1) * D], bsl,
                op=OP.subtract)

        bc_ps = bcp.tile([P, 2 * H * D], f32)
        nc.tensor.matmul(bc_ps[:], lhsT=onesr_sb[:], rhs=ab[:],
                         start=True, stop=True)
        abb = smal.tile([P, 2 * H * D], f32)
        nc.vector.tensor_copy(abb[:], bc_ps[:])
        a_v = abb[:, 0:hd].rearrange("p (h d) -> p h d", h=H)
        b_v = abb[:, hd : 2 * hd].rearrange("p (h d) -> p h d", h=H)

        # ---- per-tile output: relu((agg*A + B)) * norm_dst ----
        for t in range(n_tiles):
            agg_b = (
                agg_all[:, t * D : (t + 1) * D]
                .unsqueeze(1)
                .to_broadcast((P, H, D))
            )
            y1 = outp.tile([P, H * D], f32)
            y1v = y1[:].rearrange("p (h d) -> p h d", h=H)
            nc.vector.tensor_tensor(y1v, agg_b, a_v, op=OP.mult)
            nc.vector.tensor_tensor(y1v, y1v, b_v, op=OP.add)
            yo = outp.tile([P, H * D], f32)
            if t % 2 == 0:
                nc.scalar.activation(yo[:], y1[:], AF.Relu,
                                     scale=nd_sb[:, t : t + 1])
            else:
                nc.vector.tensor_scalar(
                    yo[:], y1[:], 0.0, nd_sb[:, t : t + 1],
                    op0=OP.max, op1=OP.mult)
            nc.sync.dma_start(out_d[t * P : (t + 1) * P, :], yo[:])

    nc.compile()
    return nc


def _get_nc(meta):
    key = (meta["n_chunks"], meta["groups"], meta["n"], meta["n_tiles"])
    if key not in _BUILD_CACHE:
        _BUILD_CACHE[key] = _build_nc(meta)
    return _BUILD_CACHE[key]


def run(h, norm, gamma, beta, src, dst, n_tiles=49, trace=False, trim=False):
    from concourse.bass_utils import run_bass_kernel_spmd

    in_maps, meta = _host_prep(h, norm, gamma, beta, src, dst, n_tiles, trim=trim)
    nc = _get_nc(meta)
    res = run_bass_kernel_spmd(
        nc, in_maps, core_ids=list(range(N_CORES)), trace=trace
    )
    n = meta["n"]
    n_tiles = meta["n_tiles"]
    shards = [np.asarray(res.results[c]["out"]) for c in range(N_CORES)]
    assign_core = meta["assign_core"]
    assign_slot = meta["assign_slot"]
    n_gtiles = N_CORES * n_tiles
    h_out = np.empty((n_gtiles * P, H * D), np.float32)
    for g in range(n_gtiles):
        sl = assign_slot[g]
        h_out[g * P : (g + 1) * P] = shards[assign_core[g]][sl * P : (sl + 1) * P]
    return h_out[:n], res


def kernel(h, e, norm, gamma, beta, src, dst):
    h_out, _ = run(h, norm, gamma, beta, src, dst)
    return h_out, e


# revision 18
# speedup vs baseline: 1.0263x; 1.0263x over previous
"""GNN message-passing layer (GAT-isotropic) on 8 Trainium2 NeuronCores.

Distribution: dst-nodes sharded across the 8 cores (graph partitioning by
destination); each core gathers remote src features directly from its full
DRAM copy of h (the halo exchange is subsumed by replicating h, which is
small), aggregates per 128-node dst tile with one-hot matmuls into PSUM,
all-reduces the batchnorm statistics across cores, and writes its output
shard. BN scale/shift are folded into per-head affine coefficients.
"""

import contextlib
import ctypes
import sys
import types

import numpy as np

# ---------------------------------------------------------------- axon shim
_SO_PATH = "/opt/axon/libaxon_pjrt.so"


def _make_ntff_hook():
    try:
        lib = ctypes.CDLL(_SO_PATH)
    except OSError:
        return None
    if not hasattr(lib, "axon_start_nrt_profile"):
        return None
    lib.axon_start_nrt_profile.argtypes = [
        ctypes.POINTER(ctypes.c_int64),
        ctypes.c_size_t,
    ]
    lib.axon_start_nrt_profile.restype = ctypes.c_int64
    lib.axon_stop_nrt_profile.argtypes = [ctypes.c_char_p]
    lib.axon_stop_nrt_profile.restype = ctypes.c_int64

    @contextlib.contextmanager
    def _hook(output_dir, device_ids):
        import jax

        jax.devices()
        if device_ids:
            ids = (ctypes.c_int64 * len(device_ids))(*device_ids)
            rc = lib.axon_start_nrt_profile(ids, len(device_ids))
        else:
            rc = lib.axon_start_nrt_profile(None, 0)
        if rc != 0:
            raise RuntimeError(f"axon_start_nrt_profile rc={rc}")
        try:
            yield
        finally:
            n = lib.axon_stop_nrt_profile(str(output_dir).encode())
            print(f"ntff profile: {n} file(s) -> {output_dir}", file=sys.stderr)

    return _hook


def _install_hook_shim():
    if "antenv.axon_hooks" in sys.modules:
        return
    try:
        import antenv
    except ImportError:
        return
    mod = types.ModuleType("antenv.axon_hooks")
    hook = _make_ntff_hook()
    mod.get_axon_ntff_profile_hook = lambda: hook
    mod.set_axon_ntff_profile_hook = lambda h: None
    sys.modules["antenv.axon_hooks"] = mod
    antenv.axon_hooks = mod


_install_hook_shim()

# ---------------------------------------------------------------- constants
P = 128
D = 64
H = 4
N_CORES = 8
EPS = 1e-5
GCH = 8  # chunks per dma_gather group (1024 idxs = SWDGE ring limit)

_BUILD_CACHE = {}


def _host_prep(h, norm, gamma, beta, src, dst, n_tiles, trim=False):
    """Shard + pad edges, build all per-core device input arrays."""
    n = h.shape[0]
    nc_nodes = n_tiles * P
    nodes_pad = N_CORES * nc_nodes
    npairs = nodes_pad // 2
    assert npairs - 1 < 2**15

    src = np.asarray(src).astype(np.int64)
    dst = np.asarray(dst).astype(np.int64)
    e = src.shape[0]
    normf = np.asarray(norm, np.float32).reshape(-1)

    g_tile = dst // P  # global dst tile
    n_gtiles = N_CORES * n_tiles
    gt_cnt = np.bincount(g_tile, minlength=n_gtiles)
    order_t = np.argsort(-gt_cnt, kind="stable")
    assign_core = np.empty(n_gtiles, np.int64)
    assign_slot = np.empty(n_gtiles, np.int64)
    assign_core[order_t] = np.arange(n_gtiles) % N_CORES
    assign_slot[order_t] = np.arange(n_gtiles) // N_CORES
    core = assign_core[g_tile]
    tile_ic = assign_slot[g_tile]
    hlf = (src & 1).astype(np.int64)  # parity of src selects pair half
    dloc = (dst % P).astype(np.float32)

    key = (core * n_tiles + tile_ic) * 2 + hlf
    order = np.argsort(key, kind="stable")
    key_s = key[order]
    counts = np.bincount(key, minlength=N_CORES * n_tiles * 2)
    n_chunks = np.ceil(
        counts.reshape(N_CORES, n_tiles, 2).max(axis=0) / P
    ).astype(np.int64)
    n_chunks = np.maximum(n_chunks, 1)  # [n_tiles, 2]

    # chunk stream order: (tile, parity) lexicographic
    flat_counts = n_chunks.reshape(-1)  # [t0p0, t0p1, t1p0, ...]
    ct = int(flat_counts.sum())
    c0 = ct  # unused (kept for meta compat)
    cum01 = np.concatenate([[0], np.cumsum(flat_counts)])[:-1]  # start chunk of (t,p)

    totslot = ct * P
    pad_idx = -1 if trim else 0
    idx_all = np.full((N_CORES, totslot), pad_idx, np.int16)
    dloc_all = np.full((N_CORES, totslot), -1.0, np.float32)
    w_all = np.zeros((N_CORES, totslot), np.float32)

    grp_start = np.concatenate([[0], np.cumsum(counts)])
    pos_in_grp = np.arange(e) - grp_start[key_s]
    c_s = core[order]
    t_s = tile_ic[order]
    h_s = hlf[order]
    src_s = src[order]
    slot_base_chunks = np.where(h_s == 0, cum01[2 * t_s], cum01[2 * t_s + 1])
    slot = slot_base_chunks * P + pos_in_grp
    idx_all[c_s, slot] = (src_s >> 1).astype(np.int16)
    dloc_all[c_s, slot] = dloc[order]
    w_all[c_s, slot] = normf[src_s]

    import ml_dtypes

    # host-built one-hot: oh[slot, j] = w[slot] * (j == dloc[slot]);
    # DRAM layout [P, ct*P] with [p, ch*P + j] for edge slot ch*P+p
    oh_cols = []
    for c in range(N_CORES):
        oh = np.zeros((totslot, P), np.float32)
        valid = dloc_all[c] >= 0
        rows = np.nonzero(valid)[0]
        oh[rows, dloc_all[c][rows].astype(np.int64)] = w_all[c][rows]
        oh_cols.append(np.ascontiguousarray(
            oh.reshape(ct, P, P).transpose(1, 0, 2).reshape(P, ct * P)
            .astype(ml_dtypes.bfloat16)))

    # gather groups: plain GCH-chunk stripes over the chunk stream
    groups = []  # (has_pads, start_chunk, n_chunks, idx_col_off)
    off = 0
    gs = 0
    while gs < ct:
        gn = min(GCH, ct - gs)
        groups.append((0, gs, gn, off))
        off += gn * P // 16
        gs += gn
    idx_cols = off

    idx_sb_np = []
    for c in range(N_CORES):
        blocks = []
        for s, gs, gn, _ in groups:
            flat = idx_all[c, gs * P : (gs + gn) * P]
            wr = flat.reshape(gn * P // 16, 16).T  # [16, gn*8]
            blocks.append(np.tile(wr, (8, 1)))  # [128, gn*8]
        idx_sb_np.append(np.concatenate(blocks, axis=1))

    # norm of dst nodes, [P, n_tiles] per core (tile slots per assignment)
    norm_pad = np.zeros(nodes_pad, np.float32)
    norm_pad[:n] = normf
    ndst = []
    for c in range(N_CORES):
        nd = np.zeros((P, n_tiles), np.float32)
        for g in range(n_gtiles):
            if assign_core[g] == c:
                nd[:, assign_slot[g]] = norm_pad[g * P : (g + 1) * P]
        ndst.append(nd)

    h_pad = np.zeros((nodes_pad, D), np.float32)
    h_pad[:n] = np.asarray(h, np.float32)
    h_pair = np.ascontiguousarray(
        h_pad.reshape(npairs, 2 * D).astype(ml_dtypes.bfloat16))

    onesc = np.ones((P, 1), np.float32)
    onesr = np.ones((1, P), np.float32)
    gbcat = np.concatenate(
        [np.asarray(gamma, np.float32).reshape(1, H * D),
         np.asarray(beta, np.float32).reshape(1, H * D)],
        axis=1,
    )

    in_maps = []
    for c in range(N_CORES):
        in_maps.append(
            {
                "hp": h_pair,
                "idx": idx_sb_np[c],
                "oh": oh_cols[c],
                "ndst": ndst[c],
                "onesc": onesc,
                "onesr": onesr,
                "gbcat": gbcat,
            }
        )

    meta = {
        "n_tiles": n_tiles,
        "n": n,
        "n_chunks": tuple(map(tuple, n_chunks.tolist())),
        "groups": tuple(groups),
        "ct": ct,
        "idx_cols": idx_cols,
        "cum01": tuple(cum01.tolist()),
        "nodes_pad": nodes_pad,
        "npairs": npairs,
        "trim": bool(trim),
        "assign_core": tuple(assign_core.tolist()),
        "assign_slot": tuple(assign_slot.tolist()),
    }
    return in_maps, meta


def _build_nc(meta):
    from contextlib import ExitStack

    import concourse.tile as tile
    from concourse import bacc, library_config, mybir

    f32 = mybir.dt.float32
    i16 = mybir.dt.int16
    AF = mybir.ActivationFunctionType
    OP = mybir.AluOpType

    bf16 = mybir.dt.bfloat16

    n_tiles = meta["n_tiles"]
    n = meta["n"]
    n_chunks = meta["n_chunks"]
    groups = meta["groups"]
    ct = meta["ct"]
    idx_cols = meta["idx_cols"]
    cum01 = meta["cum01"]
    npairs = meta["npairs"]
    nc_nodes = n_tiles * P
    NQ = 4

    nc = bacc.Bacc("TRN2", target_bir_lowering=False, debug=False,
                   num_devices=N_CORES, num_swdge_queues=NQ)

    h_d = nc.dram_tensor("hp", [npairs, 2 * D], bf16, kind="ExternalInput").ap()
    idx_d = nc.dram_tensor("idx", [P, idx_cols], i16, kind="ExternalInput").ap()
    oh_d = nc.dram_tensor("oh", [P, ct * P], bf16, kind="ExternalInput").ap()
    nd_d = nc.dram_tensor("ndst", [P, n_tiles], f32, kind="ExternalInput").ap()
    onesc_d = nc.dram_tensor("onesc", [P, 1], f32, kind="ExternalInput").ap()
    onesr_d = nc.dram_tensor("onesr", [1, P], f32, kind="ExternalInput").ap()
    gb_d = nc.dram_tensor("gbcat", [1, 2 * H * D], f32, kind="ExternalInput").ap()
    out_d = nc.dram_tensor("out", [nc_nodes, H * D], f32, kind="ExternalOutput").ap()
    cc_in = nc.dram_tensor("cc_in", [1, 2 * D], f32, kind="Internal").ap()
    cc_out = nc.dram_tensor("cc_out", [1, 2 * D], f32, kind="Internal",
                            addr_space="Shared").ap()

    # group lookup: for a global chunk id, which (group index) covers it
    grp_of_chunk = {}
    for gi, (s, gs, gn, off) in enumerate(groups):
        for j in range(gn):
            grp_of_chunk[gs + j] = gi

    with tile.TileContext(nc) as tc, ExitStack() as ctx:
        const = ctx.enter_context(tc.tile_pool(name="const", bufs=1))
        msgp = ctx.enter_context(tc.tile_pool(name="msg", bufs=32))
        ohp = ctx.enter_context(tc.tile_pool(name="oh", bufs=20))
        psp = ctx.enter_context(tc.tile_pool(name="psum", bufs=4, space="PSUM"))
        stp = ctx.enter_context(tc.tile_pool(name="stps", bufs=1, space="PSUM"))
        bcp = ctx.enter_context(tc.tile_pool(name="bc", bufs=1, space="PSUM"))
        aggp = ctx.enter_context(tc.tile_pool(name="agg", bufs=1))
        sqp = ctx.enter_context(tc.tile_pool(name="sq", bufs=2))
        outp = ctx.enter_context(tc.tile_pool(name="outp", bufs=3))
        smal = ctx.enter_context(tc.tile_pool(name="small", bufs=1))

        nc.gpsimd.load_library(library_config.mlp)

        idx_sb = const.tile([P, idx_cols], i16)
        qcol = (idx_cols // 4 + 15) & ~15
        for qi in range(4):
            lo = qi * qcol
            hi = min(idx_cols, lo + qcol)
            if lo < hi:
                nc.sync.dma_start(idx_sb[:, lo:hi], idx_d[:, lo:hi])
        nd_sb = const.tile([P, n_tiles], f32)
        nc.sync.dma_start(nd_sb[:], nd_d[:])
        onesc_sb = const.tile([P, 1], f32)
        nc.sync.dma_start(onesc_sb[:], onesc_d[:])
        onesr_sb = const.tile([1, P], f32)
        nc.sync.dma_start(onesr_sb[:], onesr_d[:])
        gb_sb = const.tile([1, 2 * H * D], f32)
        nc.sync.dma_start(gb_sb[:], gb_d[:])

        agg_all = aggp.tile([P, n_tiles * D], f32)
        sum_ps = stp.tile([1, D], f32)
        sq_ps = stp.tile([1, D], f32)

        emitted = {}

        def get_msg(chg):
            gi = grp_of_chunk[chg]
            if gi not in emitted:
                has_pads, gs, gn, off = groups[gi]
                m = msgp.tile([P, GCH, 2 * D], bf16, tag="msg")
                if has_pads and meta["trim"]:
                    # trailing idxs may be trimmed (-1): zero the tile so
                    # untouched slots stay finite (one-hot zeroes them out)
                    nc.vector.memset(m[:, :gn, :], 0.0)
                nc.gpsimd.dma_gather(
                    out_ap=m[:, :gn, :],
                    in_ap=h_d[:],
                    idxs_ap=idx_sb[:, off : off + gn * P // 16],
                    num_idxs=gn * P,
                    num_idxs_reg=gn * P,
                    elem_size=2 * D,
                    queue_num=gi % NQ,
                )
                o = ohp.tile([P, GCH * P], bf16, tag="oht")
                nc.scalar.dma_start(o[:, : gn * P], oh_d[:, gs * P : (gs + gn) * P])
                emitted[gi] = (m, o)
            has_pads, gs, gn, off = groups[gi]
            return emitted[gi], chg - gs

        for t in range(n_tiles):
            agg_ps = psp.tile([P, D], f32)
            nmm = n_chunks[t][0] + n_chunks[t][1]
            k = 0
            for hh in (0, 1):
                base = cum01[2 * t + hh]
                for j in range(n_chunks[t][hh]):
                    chg = base + j
                    (m, o), slot = get_msg(chg)
                    nc.tensor.matmul(
                        agg_ps[:], lhsT=o[:, slot * P : (slot + 1) * P],
                        rhs=m[:, slot, hh * D : (hh + 1) * D],
                        start=(k == 0), stop=(k == nmm - 1),
                    )
                    k += 1
            a_sl = agg_all[:, t * D : (t + 1) * D]
            nc.vector.tensor_copy(a_sl, agg_ps[:])
            sq = sqp.tile([P, D], f32)
            nc.vector.tensor_tensor(sq[:], a_sl, a_sl, op=OP.mult)
            nc.tensor.matmul(sum_ps[:], lhsT=onesc_sb[:], rhs=a_sl,
                             start=(t == 0), stop=(t == n_tiles - 1),
                             skip_group_check=True)
            nc.tensor.matmul(sq_ps[:], lhsT=onesc_sb[:], rhs=sq[:],
                             start=(t == 0), stop=(t == n_tiles - 1),
                             skip_group_check=True)

        # ---- batchnorm statistics all-reduce ----
        st_sb = smal.tile([1, 2 * D], f32)
        nc.scalar.activation(st_sb[:, 0:D], sum_ps[:], AF.Copy)
        nc.scalar.activation(st_sb[:, D : 2 * D], sq_ps[:], AF.Copy)
        nc.sync.dma_start(cc_in[:], st_sb[:])
        nc.gpsimd.collective_compute(
            "AllReduce", OP.add,
            replica_groups=[list(range(N_CORES))],
            ins=[cc_in[:]], outs=[cc_out[:]],
        )
        gs_sb = smal.tile([1, 2 * D], f32)
        nc.sync.dma_start(gs_sb[:], cc_out[:])

        mean = smal.tile([1, D], f32)
        nc.vector.tensor_scalar_mul(mean[:], gs_sb[:, 0:D], 1.0 / n)
        ex2 = smal.tile([1, D], f32)
        nc.vector.tensor_scalar_mul(ex2[:], gs_sb[:, D : 2 * D], 1.0 / n)
        var = smal.tile([1, D], f32)
        nc.vector.tensor_tensor(var[:], mean[:], mean[:], op=OP.mult)
        nc.vector.tensor_tensor(var[:], ex2[:], var[:], op=OP.subtract)
        nc.vector.tensor_scalar_add(var[:], var[:], EPS)
        sd = smal.tile([1, D], f32)
        nc.scalar.activation(sd[:], var[:], AF.Sqrt)
        rstd = smal.tile([1, D], f32)
        nc.vector.reciprocal(rstd[:], sd[:])

        ab = smal.tile([1, 2 * H * D], f32)
        for hh in range(H):
            asl = ab[:, hh * D : (hh + 1) * D]
            nc.vector.tensor_tensor(asl, gb_sb[:, hh * D : (hh + 1) * D],
                                    rstd[:], op=OP.mult)
        hd = H * D
        for hh in range(H):
            bsl = ab[:, hd + hh * D : hd + (hh + 1) * D]
            nc.vector.tensor_tensor(bsl, ab[:, hh * D : (hh + 1) * D],
                                    mean[:], op=OP.mult)
            nc.vector.tensor_tensor(
                bsl, gb_sb[:, hd + hh * D : hd + (hh + 1) * D], bsl,
                op=OP.subtract)

        bc_ps = bcp.tile([P, 2 * H * D], f32)
        nc.tensor.matmul(bc_ps[:], lhsT=onesr_sb[:], rhs=ab[:],
                         start=True, stop=True)
        abb = smal.tile([P, 2 * H * D], f32)
        nc.vector.tensor_copy(abb[:], bc_ps[:])
        a_v = abb[:, 0:hd].rearrange("p (h d) -> p h d", h=H)
        b_v = abb[:, hd : 2 * hd].rearrange("p (h d) -> p h d", h=H)

        # ---- output: relu((agg*A + B)) * norm_dst, two tiles per op ----
        a_v2 = abb[:, 0:hd].rearrange("p (h d) -> p h d", h=H).unsqueeze(1) \
            .to_broadcast((P, 2, H, D))
        b_v2 = abb[:, hd : 2 * hd].rearrange("p (h d) -> p h d", h=H) \
            .unsqueeze(1).to_broadcast((P, 2, H, D))
        t = 0
        while t < n_tiles:
            nt2 = 2 if t + 1 < n_tiles else 1
            agg_b = (
                agg_all[:, t * D : (t + nt2) * D]
                .rearrange("p (u d) -> p u d", u=nt2)
                .unsqueeze(2)
                .to_broadcast((P, nt2, H, D))
            )
            av = a_v2 if nt2 == 2 else a_v.unsqueeze(1).to_broadcast((P, 1, H, D))
            bv = b_v2 if nt2 == 2 else b_v.unsqueeze(1).to_broadcast((P, 1, H, D))
            y1 = outp.tile([P, 2 * H * D], f32)
            y1v = y1[:, : nt2 * H * D].rearrange(
                "p (u h d) -> p u h d", u=nt2, h=H)
            nc.vector.tensor_tensor(y1v, agg_b, av, op=OP.mult)
            nc.vector.tensor_tensor(y1v, y1v, bv, op=OP.add)
            yo = outp.tile([P, 2 * H * D], f32)
            for u in range(nt2):
                sl = slice(u * H * D, (u + 1) * H * D)
                if (t + u) % 2 == 0:
                    nc.scalar.activation(yo[:, sl], y1[:, sl], AF.Relu,
                                         scale=nd_sb[:, t + u : t + u + 1])
                else:
                    nc.vector.tensor_scalar(
                        yo[:, sl], y1[:, sl], 0.0, nd_sb[:, t + u : t + u + 1],
                        op0=OP.max, op1=OP.mult)
                nc.sync.dma_start(out_d[(t + u) * P : (t + u + 1) * P, :],
                                  yo[:, sl])
            t += nt2

    nc.compile()
    return nc


def _get_nc(meta):
    key = (meta["n_chunks"], meta["groups"], meta["n"], meta["n_tiles"])
    if key not in _BUILD_CACHE:
        _BUILD_CACHE[key] = _build_nc(meta)
    return _BUILD_CACHE[key]


def run(h, norm, gamma, beta, src, dst, n_tiles=49, trace=False, trim=False):
    from concourse.bass_utils import run_bass_kernel_spmd

    in_maps, meta = _host_prep(h, norm, gamma, beta, src, dst, n_tiles, trim=trim)
    nc = _get_nc(meta)
    res = run_bass_kernel_spmd(
        nc, in_maps, core_ids=list(range(N_CORES)), trace=trace
    )
    n = meta["n"]
    n_tiles = meta["n_tiles"]
    shards = [np.asarray(res.results[c]["out"]) for c in range(N_CORES)]
    assign_core = meta["assign_core"]
    assign_slot = meta["assign_slot"]
    n_gtiles = N_CORES * n_tiles
    h_out = np.empty((n_gtiles * P, H * D), np.float32)
    for g in range(n_gtiles):
        sl = assign_slot[g]
        h_out[g * P : (g + 1) * P] = shards[assign_core[g]][sl * P : (sl + 1) * P]
    return h_out[:n], res


def kernel(h, e, norm, gamma, beta, src, dst):
    h_out, _ = run(h, norm, gamma, beta, src, dst)
    return h_out, e


# revision 19
# speedup vs baseline: 1.0874x; 1.0595x over previous
"""GNN message-passing layer (GAT-isotropic) on 8 Trainium2 NeuronCores.

Distribution: dst-nodes sharded across the 8 cores (graph partitioning by
destination); each core gathers remote src features directly from its full
DRAM copy of h (the halo exchange is subsumed by replicating h, which is
small), aggregates per 128-node dst tile with one-hot matmuls into PSUM,
all-reduces the batchnorm statistics across cores, and writes its output
shard. BN scale/shift are folded into per-head affine coefficients.
"""

import contextlib
import ctypes
import sys
import types

import numpy as np

# ---------------------------------------------------------------- axon shim
_SO_PATH = "/opt/axon/libaxon_pjrt.so"


def _make_ntff_hook():
    try:
        lib = ctypes.CDLL(_SO_PATH)
    except OSError:
        return None
    if not hasattr(lib, "axon_start_nrt_profile"):
        return None
    lib.axon_start_nrt_profile.argtypes = [
        ctypes.POINTER(ctypes.c_int64),
        ctypes.c_size_t,
    ]
    lib.axon_start_nrt_profile.restype = ctypes.c_int64
    lib.axon_stop_nrt_profile.argtypes = [ctypes.c_char_p]
    lib.axon_stop_nrt_profile.restype = ctypes.c_int64

    @contextlib.contextmanager
    def _hook(output_dir, device_ids):
        import jax

        jax.devices()
        if device_ids:
            ids = (ctypes.c_int64 * len(device_ids))(*device_ids)
            rc = lib.axon_start_nrt_profile(ids, len(device_ids))
        else:
            rc = lib.axon_start_nrt_profile(None, 0)
        if rc != 0:
            raise RuntimeError(f"axon_start_nrt_profile rc={rc}")
        try:
            yield
        finally:
            n = lib.axon_stop_nrt_profile(str(output_dir).encode())
            print(f"ntff profile: {n} file(s) -> {output_dir}", file=sys.stderr)

    return _hook


def _install_hook_shim():
    if "antenv.axon_hooks" in sys.modules:
        return
    try:
        import antenv
    except ImportError:
        return
    mod = types.ModuleType("antenv.axon_hooks")
    hook = _make_ntff_hook()
    mod.get_axon_ntff_profile_hook = lambda: hook
    mod.set_axon_ntff_profile_hook = lambda h: None
    sys.modules["antenv.axon_hooks"] = mod
    antenv.axon_hooks = mod


_install_hook_shim()

# ---------------------------------------------------------------- constants
P = 128
D = 64
H = 4
N_CORES = 8
EPS = 1e-5
GCH = 8  # chunks per dma_gather group (1024 idxs = SWDGE ring limit)

_BUILD_CACHE = {}


def _host_prep(h, norm, gamma, beta, src, dst, n_tiles, trim=False):
    """Shard + pad edges, build all per-core device input arrays."""
    n = h.shape[0]
    nc_nodes = n_tiles * P
    nodes_pad = N_CORES * nc_nodes
    npairs = nodes_pad // 2
    assert npairs - 1 < 2**15

    src = np.asarray(src).astype(np.int64)
    dst = np.asarray(dst).astype(np.int64)
    e = src.shape[0]
    normf = np.asarray(norm, np.float32).reshape(-1)

    g_tile = dst // P  # global dst tile
    n_gtiles = N_CORES * n_tiles
    gt_cnt = np.bincount(g_tile, minlength=n_gtiles)
    order_t = np.argsort(-gt_cnt, kind="stable")
    assign_core = np.empty(n_gtiles, np.int64)
    assign_slot = np.empty(n_gtiles, np.int64)
    assign_core[order_t] = np.arange(n_gtiles) % N_CORES
    assign_slot[order_t] = np.arange(n_gtiles) // N_CORES
    core = assign_core[g_tile]
    tile_ic = assign_slot[g_tile]
    hlf = (src & 1).astype(np.int64)  # parity of src selects pair half
    dloc = (dst % P).astype(np.float32)

    key = (core * n_tiles + tile_ic) * 2 + hlf
    order = np.argsort(key, kind="stable")
    key_s = key[order]
    counts = np.bincount(key, minlength=N_CORES * n_tiles * 2)
    n_chunks = np.ceil(
        counts.reshape(N_CORES, n_tiles, 2).max(axis=0) / P
    ).astype(np.int64)
    n_chunks = np.maximum(n_chunks, 1)  # [n_tiles, 2]

    # chunk stream order: (tile, parity) lexicographic
    flat_counts = n_chunks.reshape(-1)  # [t0p0, t0p1, t1p0, ...]
    ct = int(flat_counts.sum())
    c0 = ct  # unused (kept for meta compat)
    cum01 = np.concatenate([[0], np.cumsum(flat_counts)])[:-1]  # start chunk of (t,p)

    totslot = ct * P
    pad_idx = -1 if trim else 0
    idx_all = np.full((N_CORES, totslot), pad_idx, np.int16)
    dloc_all = np.full((N_CORES, totslot), -1.0, np.float32)
    w_all = np.zeros((N_CORES, totslot), np.float32)

    grp_start = np.concatenate([[0], np.cumsum(counts)])
    pos_in_grp = np.arange(e) - grp_start[key_s]
    c_s = core[order]
    t_s = tile_ic[order]
    h_s = hlf[order]
    src_s = src[order]
    slot_base_chunks = np.where(h_s == 0, cum01[2 * t_s], cum01[2 * t_s + 1])
    slot = slot_base_chunks * P + pos_in_grp
    idx_all[c_s, slot] = (src_s >> 1).astype(np.int16)
    dloc_all[c_s, slot] = dloc[order]
    w_all[c_s, slot] = normf[src_s]

    import ml_dtypes

    # host-built one-hot: oh[slot, j] = w[slot] * (j == dloc[slot]);
    # DRAM layout [P, ct*P] with [p, ch*P + j] for edge slot ch*P+p
    oh_cols = []
    for c in range(N_CORES):
        oh = np.zeros((totslot, P), np.float32)
        valid = dloc_all[c] >= 0
        rows = np.nonzero(valid)[0]
        oh[rows, dloc_all[c][rows].astype(np.int64)] = w_all[c][rows]
        oh_cols.append(np.ascontiguousarray(
            oh.reshape(ct, P, P).transpose(1, 0, 2).reshape(P, ct * P)
            .astype(ml_dtypes.bfloat16)))

    # gather groups: plain GCH-chunk stripes over the chunk stream
    groups = []  # (has_pads, start_chunk, n_chunks, idx_col_off)
    off = 0
    gs = 0
    while gs < ct:
        gn = min(GCH, ct - gs)
        groups.append((0, gs, gn, off))
        off += gn * P // 16
        gs += gn
    idx_cols = off

    idx_sb_np = []
    for c in range(N_CORES):
        blocks = []
        for s, gs, gn, _ in groups:
            flat = idx_all[c, gs * P : (gs + gn) * P]
            wr = flat.reshape(gn * P // 16, 16).T  # [16, gn*8]
            blocks.append(np.tile(wr, (8, 1)))  # [128, gn*8]
        idx_sb_np.append(np.concatenate(blocks, axis=1))

    # norm of dst nodes, [P, n_tiles] per core (tile slots per assignment)
    norm_pad = np.zeros(nodes_pad, np.float32)
    norm_pad[:n] = normf
    ndst = []
    for c in range(N_CORES):
        nd = np.zeros((P, n_tiles), np.float32)
        for g in range(n_gtiles):
            if assign_core[g] == c:
                nd[:, assign_slot[g]] = norm_pad[g * P : (g + 1) * P]
        ndst.append(nd)

    h_pad = np.zeros((nodes_pad, D), np.float32)
    h_pad[:n] = np.asarray(h, np.float32)
    h_pair = np.ascontiguousarray(
        h_pad.reshape(npairs, 2 * D).astype(ml_dtypes.bfloat16))

    onesc = np.ones((P, 1), np.float32)
    onesr = np.ones((1, P), np.float32)
    gbcat = np.concatenate(
        [np.asarray(gamma, np.float32).reshape(1, H * D),
         np.asarray(beta, np.float32).reshape(1, H * D)],
        axis=1,
    )

    in_maps = []
    for c in range(N_CORES):
        in_maps.append(
            {
                "hp": h_pair,
                "idx": idx_sb_np[c],
                "oh": oh_cols[c],
                "ndst": ndst[c],
                "onesc": onesc,
                "onesr": onesr,
                "gbcat": gbcat,
            }
        )

    meta = {
        "n_tiles": n_tiles,
        "n": n,
        "n_chunks": tuple(map(tuple, n_chunks.tolist())),
        "groups": tuple(groups),
        "ct": ct,
        "idx_cols": idx_cols,
        "cum01": tuple(cum01.tolist()),
        "nodes_pad": nodes_pad,
        "npairs": npairs,
        "trim": bool(trim),
        "assign_core": tuple(assign_core.tolist()),
        "assign_slot": tuple(assign_slot.tolist()),
    }
    return in_maps, meta


def _build_nc(meta):
    from contextlib import ExitStack

    import concourse.tile as tile
    from concourse import bacc, library_config, mybir

    f32 = mybir.dt.float32
    i16 = mybir.dt.int16
    AF = mybir.ActivationFunctionType
    OP = mybir.AluOpType

    bf16 = mybir.dt.bfloat16

    n_tiles = meta["n_tiles"]
    n = meta["n"]
    n_chunks = meta["n_chunks"]
    groups = meta["groups"]
    ct = meta["ct"]
    idx_cols = meta["idx_cols"]
    cum01 = meta["cum01"]
    npairs = meta["npairs"]
    nc_nodes = n_tiles * P
    NQ = 4

    nc = bacc.Bacc("TRN2", target_bir_lowering=False, debug=False,
                   num_devices=N_CORES, num_swdge_queues=NQ)

    h_d = nc.dram_tensor("hp", [npairs, 2 * D], bf16, kind="ExternalInput").ap()
    idx_d = nc.dram_tensor("idx", [P, idx_cols], i16, kind="ExternalInput").ap()
    oh_d = nc.dram_tensor("oh", [P, ct * P], bf16, kind="ExternalInput").ap()
    nd_d = nc.dram_tensor("ndst", [P, n_tiles], f32, kind="ExternalInput").ap()
    onesc_d = nc.dram_tensor("onesc", [P, 1], f32, kind="ExternalInput").ap()
    onesr_d = nc.dram_tensor("onesr", [1, P], f32, kind="ExternalInput").ap()
    gb_d = nc.dram_tensor("gbcat", [1, 2 * H * D], f32, kind="ExternalInput").ap()
    out_d = nc.dram_tensor("out", [nc_nodes, H * D], f32, kind="ExternalOutput").ap()
    cc_in = nc.dram_tensor("cc_in", [1, 2 * D], f32, kind="Internal").ap()
    cc_out = nc.dram_tensor("cc_out", [1, 2 * D], f32, kind="Internal",
                            addr_space="Shared").ap()

    # group lookup: for a global chunk id, which (group index) covers it
    grp_of_chunk = {}
    for gi, (s, gs, gn, off) in enumerate(groups):
        for j in range(gn):
            grp_of_chunk[gs + j] = gi

    with tile.TileContext(nc) as tc, ExitStack() as ctx:
        const = ctx.enter_context(tc.tile_pool(name="const", bufs=1))
        msgp = ctx.enter_context(tc.tile_pool(name="msg", bufs=32))
        ohp = ctx.enter_context(tc.tile_pool(name="oh", bufs=20))
        psp = ctx.enter_context(tc.tile_pool(name="psum", bufs=4, space="PSUM"))
        stp = ctx.enter_context(tc.tile_pool(name="stps", bufs=1, space="PSUM"))
        bcp = ctx.enter_context(tc.tile_pool(name="bc", bufs=1, space="PSUM"))
        aggp = ctx.enter_context(tc.tile_pool(name="agg", bufs=1))
        sqp = ctx.enter_context(tc.tile_pool(name="sq", bufs=2))
        outp = ctx.enter_context(tc.tile_pool(name="outp", bufs=3))
        smal = ctx.enter_context(tc.tile_pool(name="small", bufs=1))

        nc.gpsimd.load_library(library_config.mlp)

        idx_sb = const.tile([P, idx_cols], i16)
        qcol = (idx_cols // 8 + 15) & ~15
        for qi in range(8):
            lo = qi * qcol
            hi = min(idx_cols, lo + qcol)
            if lo < hi:
                nc.sync.dma_start(idx_sb[:, lo:hi], idx_d[:, lo:hi])
        nd_sb = const.tile([P, n_tiles], f32)
        nc.sync.dma_start(nd_sb[:], nd_d[:])
        onesc_sb = const.tile([P, 1], f32)
        nc.sync.dma_start(onesc_sb[:], onesc_d[:])
        onesr_sb = const.tile([1, P], f32)
        nc.sync.dma_start(onesr_sb[:], onesr_d[:])
        gb_sb = const.tile([1, 2 * H * D], f32)
        nc.sync.dma_start(gb_sb[:], gb_d[:])

        agg_all = aggp.tile([P, n_tiles * D], f32)
        sum_ps = stp.tile([1, D], f32)
        sq_ps = stp.tile([1, D], f32)

        emitted = {}

        def get_msg(chg):
            gi = grp_of_chunk[chg]
            if gi not in emitted:
                has_pads, gs, gn, off = groups[gi]
                m = msgp.tile([P, GCH, 2 * D], bf16, tag="msg")
                if has_pads and meta["trim"]:
                    # trailing idxs may be trimmed (-1): zero the tile so
                    # untouched slots stay finite (one-hot zeroes them out)
                    nc.vector.memset(m[:, :gn, :], 0.0)
                nc.gpsimd.dma_gather(
                    out_ap=m[:, :gn, :],
                    in_ap=h_d[:],
                    idxs_ap=idx_sb[:, off : off + gn * P // 16],
                    num_idxs=gn * P,
                    num_idxs_reg=gn * P,
                    elem_size=2 * D,
                    queue_num=gi % NQ,
                )
                o = ohp.tile([P, GCH * P], bf16, tag="oht")
                nc.scalar.dma_start(o[:, : gn * P], oh_d[:, gs * P : (gs + gn) * P])
                emitted[gi] = (m, o)
            has_pads, gs, gn, off = groups[gi]
            return emitted[gi], chg - gs

        for t in range(n_tiles):
            agg_ps = psp.tile([P, D], f32)
            nmm = n_chunks[t][0] + n_chunks[t][1]
            k = 0
            for hh in (0, 1):
                base = cum01[2 * t + hh]
                for j in range(n_chunks[t][hh]):
                    chg = base + j
                    (m, o), slot = get_msg(chg)
                    nc.tensor.matmul(
                        agg_ps[:], lhsT=o[:, slot * P : (slot + 1) * P],
                        rhs=m[:, slot, hh * D : (hh + 1) * D],
                        start=(k == 0), stop=(k == nmm - 1),
                    )
                    k += 1
            a_sl = agg_all[:, t * D : (t + 1) * D]
            nc.vector.tensor_copy(a_sl, agg_ps[:])
            sq = sqp.tile([P, D], f32)
            nc.vector.tensor_tensor(sq[:], a_sl, a_sl, op=OP.mult)
            nc.tensor.matmul(sum_ps[:], lhsT=onesc_sb[:], rhs=a_sl,
                             start=(t == 0), stop=(t == n_tiles - 1),
                             skip_group_check=True)
            nc.tensor.matmul(sq_ps[:], lhsT=onesc_sb[:], rhs=sq[:],
                             start=(t == 0), stop=(t == n_tiles - 1),
                             skip_group_check=True)

        # ---- batchnorm statistics all-reduce ----
        st_sb = smal.tile([1, 2 * D], f32)
        nc.scalar.activation(st_sb[:, 0:D], sum_ps[:], AF.Copy)
        nc.scalar.activation(st_sb[:, D : 2 * D], sq_ps[:], AF.Copy)
        nc.sync.dma_start(cc_in[:], st_sb[:])
        nc.gpsimd.collective_compute(
            "AllReduce", OP.add,
            replica_groups=[list(range(N_CORES))],
            ins=[cc_in[:]], outs=[cc_out[:]],
        )
        gs_sb = smal.tile([1, 2 * D], f32)
        nc.sync.dma_start(gs_sb[:], cc_out[:])

        mean = smal.tile([1, D], f32)
        nc.vector.tensor_scalar_mul(mean[:], gs_sb[:, 0:D], 1.0 / n)
        ex2 = smal.tile([1, D], f32)
        nc.vector.tensor_scalar_mul(ex2[:], gs_sb[:, D : 2 * D], 1.0 / n)
        var = smal.tile([1, D], f32)
        nc.vector.tensor_tensor(var[:], mean[:], mean[:], op=OP.mult)
        nc.vector.tensor_tensor(var[:], ex2[:], var[:], op=OP.subtract)
        nc.vector.tensor_scalar_add(var[:], var[:], EPS)
        sd = smal.tile([1, D], f32)
        nc.scalar.activation(sd[:], var[:], AF.Sqrt)
        rstd = smal.tile([1, D], f32)
        nc.vector.reciprocal(rstd[:], sd[:])

        ab = smal.tile([1, 2 * H * D], f32)
        for hh in range(H):
            asl = ab[:, hh * D : (hh + 1) * D]
            nc.vector.tensor_tensor(asl, gb_sb[:, hh * D : (hh + 1) * D],
                                    rstd[:], op=OP.mult)
        hd = H * D
        for hh in range(H):
            bsl = ab[:, hd + hh * D : hd + (hh + 1) * D]
            nc.vector.tensor_tensor(bsl, ab[:, hh * D : (hh + 1) * D],
                                    mean[:], op=OP.mult)
            nc.vector.tensor_tensor(
                bsl, gb_sb[:, hd + hh * D : hd + (hh + 1) * D], bsl,
                op=OP.subtract)

        bc_ps = bcp.tile([P, 2 * H * D], f32)
        nc.tensor.matmul(bc_ps[:], lhsT=onesr_sb[:], rhs=ab[:],
                         start=True, stop=True)
        abb = smal.tile([P, 2 * H * D], f32)
        nc.vector.tensor_copy(abb[:], bc_ps[:])
        a_v = abb[:, 0:hd].rearrange("p (h d) -> p h d", h=H)
        b_v = abb[:, hd : 2 * hd].rearrange("p (h d) -> p h d", h=H)

        # ---- output: relu((agg*A + B)) * norm_dst, two tiles per op ----
        a_v2 = abb[:, 0:hd].rearrange("p (h d) -> p h d", h=H).unsqueeze(1) \
            .to_broadcast((P, 2, H, D))
        b_v2 = abb[:, hd : 2 * hd].rearrange("p (h d) -> p h d", h=H) \
            .unsqueeze(1).to_broadcast((P, 2, H, D))
        t = 0
        while t < n_tiles:
            nt2 = 2 if t + 1 < n_tiles else 1
            agg_b = (
                agg_all[:, t * D : (t + nt2) * D]
                .rearrange("p (u d) -> p u d", u=nt2)
                .unsqueeze(2)
                .to_broadcast((P, nt2, H, D))
            )
            av = a_v2 if nt2 == 2 else a_v.unsqueeze(1).to_broadcast((P, 1, H, D))
            bv = b_v2 if nt2 == 2 else b_v.unsqueeze(1).to_broadcast((P, 1, H, D))
            y1 = outp.tile([P, 2 * H * D], f32)
            y1v = y1[:, : nt2 * H * D].rearrange(
                "p (u h d) -> p u h d", u=nt2, h=H)
            nc.vector.tensor_tensor(y1v, agg_b, av, op=OP.mult)
            nc.vector.tensor_tensor(y1v, y1v, bv, op=OP.add)
            yo = outp.tile([P, 2 * H * D], f32)
            for u in range(nt2):
                sl = slice(u * H * D, (u + 1) * H * D)
                if (t + u) % 2 == 0:
                    nc.scalar.activation(yo[:, sl], y1[:, sl], AF.Relu,
                                         scale=nd_sb[:, t + u : t + u + 1])
                else:
                    nc.vector.tensor_scalar(
                        yo[:, sl], y1[:, sl], 0.0, nd_sb[:, t + u : t + u + 1],
                        op0=OP.max, op1=OP.mult)
                nc.sync.dma_start(out_d[(t + u) * P : (t + u + 1) * P, :],
                                  yo[:, sl])
            t += nt2

    nc.compile()
    return nc


def _get_nc(meta):
    key = (meta["n_chunks"], meta["groups"], meta["n"], meta["n_tiles"])
    if key not in _BUILD_CACHE:
        _BUILD_CACHE[key] = _build_nc(meta)
    return _BUILD_CACHE[key]


def run(h, norm, gamma, beta, src, dst, n_tiles=49, trace=False, trim=False):
    from concourse.bass_utils import run_bass_kernel_spmd

    in_maps, meta = _host_prep(h, norm, gamma, beta, src, dst, n_tiles, trim=trim)
    nc = _get_nc(meta)
    res = run_bass_kernel_spmd(
        nc, in_maps, core_ids=list(range(N_CORES)), trace=trace
    )
    n = meta["n"]
    n_tiles = meta["n_tiles"]
    shards = [np.asarray(res.results[c]["out"]) for c in range(N_CORES)]
    assign_core = meta["assign_core"]
    assign_slot = meta["assign_slot"]
    n_gtiles = N_CORES * n_tiles
    h_out = np.empty((n_gtiles * P, H * D), np.float32)
    for g in range(n_gtiles):
        sl = assign_slot[g]
        h_out[g * P : (g + 1) * P] = shards[assign_core[g]][sl * P : (sl + 1) * P]
    return h_out[:n], res


def kernel(h, e, norm, gamma, beta, src, dst):
    h_out, _ = run(h, norm, gamma, beta, src, dst)
    return h_out, e
